# revision 6
# baseline (speedup 1.0000x reference)
"""Trainium2 Bass kernel for the MANE multi-view SGNS embedding loss.

Strategy: data-parallel over the batch axis B across 8 NeuronCores with the
embedding tables replicated per core.  The two tables are interleaved
view-major into one [2*N*V, D] fp16 DRAM tensor so that the positive-context
and center rows for all V views of one index are contiguous (one 768B
descriptor gathers all three).  Negative-row gathers run as large multi-index
SWDGE indirect DMAs (10240 rows per instruction) to amortize the ~1us fixed
descriptor-generation cost per SWDGE instruction.  Dot products run on the
vector engine in fp16 (mult) + fp32 reduce; log-sigmoid + per-term reduction
on the scalar engine (Sigmoid -> Ln with accum_out).  Per-core partial sums
[P, 2*T] are combined on the host (scalar all-reduce).
"""

import numpy as np

import concourse.bass as bass
import concourse.bacc as bacc
import concourse.tile as tile
from concourse import mybir
from concourse.bass_utils import run_bass_kernel_spmd

# ---------------------------------------------------------------- problem dims
V, N, D = 3, 200000, 128
B, K = 32768, 10
TOTAL = 65536
NCORES = 8
P = 128
T = 3 + 2 * V * (V - 1)  # 15 terms

F32 = mybir.dt.float32
F16 = mybir.dt.float16
I32 = mybir.dt.int32

# (j, i) pairs in reference order for cost2/cost3
PAIRS = [(j, i) for j in range(V) for i in range(V) if i != j]
# center view per term: cost1[i] -> i, cost2/3 (j,i) -> i
TERM_VIEW = [0, 1, 2] + [i for (_, i) in PAIRS] + [i for (_, i) in PAIRS]


def build_bass(bc, k, nchunk):
    """Build + compile the per-core Tile program.

    bc: batch elems per core; k: negatives per positive; nchunk: number of
    gather/compute chunks (each chunk covers bc//nchunk batch elems).
    """
    chunk = bc // nchunk
    jb = chunk // P               # batch elems per partition per chunk
    assert jb * P * nchunk == bc
    pc_cols = nchunk * jb         # 32
    nc_cols = nchunk * jb * k     # 320

    nc = bacc.Bacc("TRN2", target_bir_lowering=False, debug=False,
                   enable_asserts=False, num_swdge_queues=4)
    qrr = [0]

    def gather(out_ap, in_ap, off_ap):
        # one row-set per partition; round-robin the 4 SWDGE queues so
        # descriptor generation parallelizes across Q7 cores
        inst = nc.gpsimd.indirect_dma_start(
            out=out_ap, out_offset=None, in_=in_ap,
            in_offset=bass.IndirectOffsetOnAxis(ap=off_ap, axis=0))
        q = qrr[0] % 4
        qrr[0] += 1
        if q:
            inst.ins.queue = f"qPoolDynamic{q}"
        return inst

    W = nc.dram_tensor("w_all", [2 * N * V, D], F16, kind="ExternalInput")
    pnf = nc.dram_tensor("pnf", [V, P, pc_cols], I32, kind="ExternalInput")
    pgf = nc.dram_tensor("pgf", [V, P, pc_cols], I32, kind="ExternalInput")
    nidx = nc.dram_tensor("nidx", [T, P, nc_cols], I32, kind="ExternalInput")
    acc_out = nc.dram_tensor("acc", [P, 2 * T], F32, kind="ExternalOutput")

    SIG = mybir.ActivationFunctionType.Sigmoid
    LN = mybir.ActivationFunctionType.Ln
    CPY = mybir.ActivationFunctionType.Copy
    MUL = mybir.AluOpType.mult
    ADD = mybir.AluOpType.add
    AXX = mybir.AxisListType.X

    from contextlib import ExitStack
    with tile.TileContext(nc) as tc, ExitStack() as ctx:
        idx_pool = ctx.enter_context(tc.tile_pool(name="idx", bufs=1))
        pv_pool = ctx.enter_context(tc.tile_pool(name="pv", bufs=4))
        cen_pool = ctx.enter_context(tc.tile_pool(name="cen", bufs=1))
        neg_pool = ctx.enter_context(tc.tile_pool(name="neg", bufs=3))
        prod_pool = ctx.enter_context(tc.tile_pool(name="prod", bufs=1))
        red_pool = ctx.enter_context(tc.tile_pool(name="red", bufs=1))
        act_pool = ctx.enter_context(tc.tile_pool(name="act", bufs=2))
        out_pool = ctx.enter_context(tc.tile_pool(name="out", bufs=1))

        # ---- load all index tiles up front (small)
        PNF, PGF, NIX = [], [], []
        for v in range(V):
            tpn = idx_pool.tile([P, pc_cols], I32, tag=f"pnf{v}",
                                name=f"pnf{v}")
            nc.sync.dma_start(tpn[:], pnf.ap()[v])
            PNF.append(tpn)
            tpg = idx_pool.tile([P, pc_cols], I32, tag=f"pgf{v}",
                                name=f"pgf{v}")
            nc.sync.dma_start(tpg[:], pgf.ap()[v])
            PGF.append(tpg)
        for t in range(T):
            tn = idx_pool.tile([P, nc_cols], I32, tag=f"nidx{t}",
                               name=f"nidx{t}")
            nc.sync.dma_start(tn[:], nidx.ap()[t])
            NIX.append(tn)

        # ---- resident tiles
        CEN = [cen_pool.tile([P, pc_cols * D], F16, tag=f"cen{v}",
                             name=f"cen{v}") for v in range(V)]
        XN = [red_pool.tile([P, nc_cols], F32, tag=f"xn{t}", name=f"xn{t}")
              for t in range(T)]
        XPN = [red_pool.tile([P, pc_cols * V], F32, tag=f"xpn{v}",
                             name=f"xpn{v}") for v in range(V)]
        XPG = [red_pool.tile([P, pc_cols * V], F32, tag=f"xpg{v}",
                             name=f"xpg{v}") for v in range(V)]
        ACC = out_pool.tile([P, 2 * T], F32)

        for c in range(nchunk):
            jsl = slice(c * jb, (c + 1) * jb)
            for v in range(V):
                # node-fused gather: idx [P, jb] -> [P, jb*V*D]
                # (V*D contiguous elements per index: V view-rows of index n)
                PVNt = pv_pool.tile([P, jb * V * D], F16, tag="pv",
                                    name=f"pvn_{c}_{v}")
                for j in range(jb):
                    gather(PVNt[:, j * V * D:(j + 1) * V * D], W.ap(),
                           PNF[v][:, c * jb + j:c * jb + j + 1])
                pvn4 = PVNt[:].rearrange("p (j w d) -> p j w d", w=V, d=D)
                cen_src = pvn4[:, :, v, :]          # [P, jb, D] stride V*D
                csl = CEN[v][:, c * jb * D:(c + 1) * jb * D] \
                    .rearrange("p (j d) -> p j d", d=D)
                nc.scalar.activation(out=csl, in_=cen_src, func=CPY)
                # node-pos dots for all V components against center v
                prodp = prod_pool.tile([P, jb * V * D], F16, tag="prodp",
                                       name=f"prodp_{c}_{v}")
                pr4 = prodp[:].rearrange("p (j w d) -> p j w d", w=V, d=D)
                nc.vector.tensor_tensor(
                    out=pr4, in0=pvn4,
                    in1=cen_src.unsqueeze(2).to_broadcast([P, jb, V, D]),
                    op=MUL)
                nc.vector.tensor_reduce(
                    out=XPN[v][:, c * jb * V:(c + 1) * jb * V],
                    in_=pr4, axis=AXX, op=ADD)
                # neigh-fused gather + pos dots
                PVGt = pv_pool.tile([P, jb * V * D], F16, tag="pv",
                                    name=f"pvg_{c}_{v}")
                for j in range(jb):
                    gather(PVGt[:, j * V * D:(j + 1) * V * D], W.ap(),
                           PGF[v][:, c * jb + j:c * jb + j + 1])
                pvg4 = PVGt[:].rearrange("p (j w d) -> p j w d", w=V, d=D)
                prodg = prod_pool.tile([P, jb * V * D], F16, tag="prodp",
                                       name=f"prodg_{c}_{v}")
                pg4 = prodg[:].rearrange("p (j w d) -> p j w d", w=V, d=D)
                nc.vector.tensor_tensor(
                    out=pg4, in0=pvg4,
                    in1=csl.unsqueeze(2).to_broadcast([P, jb, V, D]),
                    op=MUL)
                nc.vector.tensor_reduce(
                    out=XPG[v][:, c * jb * V:(c + 1) * jb * V],
                    in_=pg4, axis=AXX, op=ADD)

            for t in range(T):
                iv = TERM_VIEW[t]
                NEG = neg_pool.tile([P, jb * k * D], F16, tag="neg",
                                    name=f"neg_{c}_{t}")
                for q in range(jb * k):
                    col = c * jb * k + q
                    gather(NEG[:, q * D:(q + 1) * D], W.ap(),
                           NIX[t][:, col:col + 1])
                neg4 = NEG[:].rearrange("p (j k d) -> p j k d", k=k, d=D)
                cenv = CEN[iv][:, c * jb * D:(c + 1) * jb * D] \
                    .rearrange("p (j d) -> p j d", d=D)
                prod = prod_pool.tile([P, jb * k * D], F16, tag="prod",
                                      name=f"prod_{c}_{t}")
                pr = prod[:].rearrange("p (j k d) -> p j k d", k=k, d=D)
                nc.vector.tensor_tensor(
                    out=pr, in0=neg4,
                    in1=cenv.unsqueeze(2).to_broadcast([P, jb, k, D]),
                    op=MUL)
                nc.vector.tensor_reduce(
                    out=XN[t][:, c * jb * k:(c + 1) * jb * k],
                    in_=pr, axis=AXX, op=ADD)

        # ---- log-sigmoid + per-term accumulation
        # neg sum: ln(sigmoid(-x)); pos sum: ln(sigmoid(x))
        for t in range(T):
            sgn = act_pool.tile([P, nc_cols], F32, tag="sgn", name=f"sgn{t}")
            nc.scalar.activation(out=sgn[:], in_=XN[t][:], func=SIG,
                                 scale=-1.0)
            spn = act_pool.tile([P, nc_cols], F32, tag="spn", name=f"spn{t}")
            nc.scalar.activation(out=spn[:], in_=sgn[:], func=LN,
                                 accum_out=ACC[:, t:t + 1])
            if t < V:
                src, col = XPG[t], t
            elif t < V + len(PAIRS):
                j, i = PAIRS[t - V]
                src, col = XPN[i], j
            else:
                j, i = PAIRS[t - V - len(PAIRS)]
                src, col = XPG[i], j
            pos_ap = src[:].rearrange("p (s w) -> p s w", w=V)[:, :, col]
            sgp = act_pool.tile([P, pc_cols], F32, tag="sgp", name=f"sgp{t}")
            nc.scalar.activation(out=sgp[:], in_=pos_ap, func=SIG)
            spp = act_pool.tile([P, pc_cols], F32, tag="spp", name=f"spp{t}")
            nc.scalar.activation(out=spp[:], in_=sgp[:], func=LN,
                                 accum_out=ACC[:, T + t:T + t + 1])

        nc.sync.dma_start(acc_out.ap(), ACC[:])

    nc.compile()
    return nc


_NC_CACHE = {}


def _get_nc(bc, k, nchunk):
    key = (bc, k, nchunk)
    if key not in _NC_CACHE:
        _NC_CACHE[key] = build_bass(bc, k, nchunk)
    return _NC_CACHE[key]


def _lay2(x, nchunk, jb):
    # x: [..., bc] -> [..., P, nchunk*jb] with (c,p,j) -> col c*jb+j on part p
    lead = x.shape[:-1]
    return (x.reshape(*lead, nchunk, P, jb)
             .swapaxes(-3, -2)
             .reshape(*lead, P, nchunk * jb))


def _lay3(x, nchunk, jb, k):
    # x: [..., bc, k] -> [..., P, nchunk*jb*k]
    lead = x.shape[:-2]
    return (x.reshape(*lead, nchunk, P, jb, k)
             .swapaxes(-4, -3)
             .reshape(*lead, P, nchunk * jb * k))


def host_prep(count, shuffle_indices, nodes_idx, neigh_idx,
              neg_idx1, neg_idx2, neg_idx3, node_W, neigh_W,
              n_cores=NCORES, nchunk=4, b=B):
    """Compute per-core input maps + the fused fp16 table. Pure numpy."""
    c0 = int(count)
    sh = np.asarray(shuffle_indices)[:, c0:c0 + b].astype(np.int64)
    nodes_sel = np.take_along_axis(
        np.asarray(nodes_idx).astype(np.int64), sh, axis=1)
    neigh_sel = np.take_along_axis(
        np.asarray(neigh_idx).astype(np.int64), sh, axis=1)
    neg1 = np.asarray(neg_idx1).astype(np.int64)[:, :b]
    neg2 = np.asarray(neg_idx2).astype(np.int64)[:, :, :b]
    neg3 = np.asarray(neg_idx3).astype(np.int64)[:, :, :b]

    # fused view-major interleave: node rows at n*V+v, neigh at N*V + n*V+v
    node16 = np.asarray(node_W).astype(np.float16)    # [V, N, D]
    neigh16 = np.asarray(neigh_W).astype(np.float16)
    W_all = np.empty((2 * N * V, D), dtype=np.float16)
    W_all[:N * V] = node16.transpose(1, 0, 2).reshape(N * V, D)
    W_all[N * V:] = neigh16.transpose(1, 0, 2).reshape(N * V, D)

    # per-term fused negative indices
    neg_list = []
    for i in range(V):
        neg_list.append(N * V + neg1[i] * V + i)
    for (j, i) in PAIRS:
        neg_list.append(neg2[j, i] * V + j)
    for (j, i) in PAIRS:
        neg_list.append(N * V + neg3[j, i] * V + j)
    neg_all = np.stack(neg_list)                      # [T, b, K]
    pnf_all = nodes_sel * V                           # [V, b]
    pgf_all = N * V + neigh_sel * V                   # [V, b]

    bc = b // n_cores
    chunk = bc // nchunk
    jb = chunk // P
    k = neg_all.shape[-1]

    in_maps = []
    for core in range(n_cores):
        sl = slice(core * bc, (core + 1) * bc)
        in_maps.append({
            "w_all": W_all,
            "pnf": _lay2(pnf_all[:, sl], nchunk, jb).astype(np.int32),
            "pgf": _lay2(pgf_all[:, sl], nchunk, jb).astype(np.int32),
            "nidx": _lay3(neg_all[:, sl], nchunk, jb, k).astype(np.int32),
        })
    return in_maps


def host_combine(acc_list, hyp1, hyp2, b=B):
    """acc_list: per-core [P, 2T] log-sigmoid-sum partials -> final scalar."""
    s = np.zeros(T, dtype=np.float64)
    for a in acc_list:
        a = np.asarray(a, dtype=np.float64).sum(axis=0)
        s += a[:T] + a[T:2 * T]
    term_val = s / b
    cost1 = term_val[0:3].mean()
    cost2 = float(np.asarray(hyp1).reshape(-1)[0]) * term_val[3:9].sum() / 6.0
    cost3 = float(np.asarray(hyp2).reshape(-1)[0]) * term_val[9:15].sum() / 6.0
    return np.array(-(cost1 + cost2 + cost3) / 3.0, dtype=np.float32)


def kernel(count, shuffle_indices, nodes_idx, neigh_idx,
           neg_idx1, neg_idx2, neg_idx3, node_W, neigh_W, hyp1, hyp2):
    in_maps = host_prep(count, shuffle_indices, nodes_idx, neigh_idx,
                        neg_idx1, neg_idx2, neg_idx3, node_W, neigh_W)
    nc = _get_nc(B // NCORES, K, 4)
    res = run_bass_kernel_spmd(nc, in_maps, core_ids=list(range(NCORES)))
    acc_list = [r["acc"] for r in res.results]
    return host_combine(acc_list, hyp1, hyp2)


# revision 16
# speedup vs baseline: 1.0074x; 1.0074x over previous
"""Trainium2 Bass kernel for the MANE multi-view SGNS embedding loss.

Strategy: data-parallel over the batch axis B across 8 NeuronCores with the
embedding tables replicated per core.  The two tables are interleaved
view-major into one [2*N*V, D] fp16 DRAM tensor so that the positive-context
and center rows for all V views of one index are contiguous (one 768B
descriptor gathers all three).  Negative-row gathers run as large multi-index
SWDGE indirect DMAs (10240 rows per instruction) to amortize the ~1us fixed
descriptor-generation cost per SWDGE instruction.  Dot products run on the
vector engine in fp16 (mult) + fp32 reduce; log-sigmoid + per-term reduction
on the scalar engine (Sigmoid -> Ln with accum_out).  Per-core partial sums
[P, 2*T] are combined on the host (scalar all-reduce).
"""

import numpy as np

import concourse.bass as bass
import concourse.bacc as bacc
import concourse.tile as tile
from concourse import mybir
from concourse.bass_utils import run_bass_kernel_spmd

# ---------------------------------------------------------------- problem dims
V, N, D = 3, 200000, 128
B, K = 32768, 10
TOTAL = 65536
NCORES = 8
P = 128
T = 3 + 2 * V * (V - 1)  # 15 terms

F32 = mybir.dt.float32
F16 = mybir.dt.float16
I32 = mybir.dt.int32

# (j, i) pairs in reference order for cost2/cost3
PAIRS = [(j, i) for j in range(V) for i in range(V) if i != j]
# center view per term: cost1[i] -> i, cost2/3 (j,i) -> i
TERM_VIEW = [0, 1, 2] + [i for (_, i) in PAIRS] + [i for (_, i) in PAIRS]


def build_bass(bc, k, nchunk):
    """Build + compile the per-core Tile program.

    bc: batch elems per core; k: negatives per positive; nchunk: number of
    gather/compute chunks (each chunk covers bc//nchunk batch elems).
    """
    chunk = bc // nchunk
    jb = chunk // P               # batch elems per partition per chunk
    assert jb * P * nchunk == bc
    pc_cols = nchunk * jb         # 32
    nc_cols = nchunk * jb * k     # 320

    nc = bacc.Bacc("TRN2", target_bir_lowering=False, debug=False,
                   enable_asserts=False, num_swdge_queues=4)
    qrr = [0]

    def gather(out_ap, in_ap, off_ap):
        # one row-set per partition; round-robin the 4 SWDGE queues so
        # descriptor generation parallelizes across Q7 cores
        inst = nc.gpsimd.indirect_dma_start(
            out=out_ap, out_offset=None, in_=in_ap,
            in_offset=bass.IndirectOffsetOnAxis(ap=off_ap, axis=0))
        q = qrr[0] % 4
        qrr[0] += 1
        if q:
            inst.ins.queue = f"qPoolDynamic{q}"
        return inst

    W = nc.dram_tensor("w_all", [2 * N * V, D], F16, kind="ExternalInput")
    pnf = nc.dram_tensor("pnf", [V, P, pc_cols], I32, kind="ExternalInput")
    pgf = nc.dram_tensor("pgf", [V, P, pc_cols], I32, kind="ExternalInput")
    nidx = nc.dram_tensor("nidx", [T, P, nc_cols], I32, kind="ExternalInput")
    acc_out = nc.dram_tensor("acc", [P, 2 * T], F32, kind="ExternalOutput")

    SIG = mybir.ActivationFunctionType.Sigmoid
    LN = mybir.ActivationFunctionType.Ln
    CPY = mybir.ActivationFunctionType.Copy
    MUL = mybir.AluOpType.mult
    ADD = mybir.AluOpType.add
    AXX = mybir.AxisListType.X

    from contextlib import ExitStack
    with tile.TileContext(nc) as tc, ExitStack() as ctx:
        idx_pool = ctx.enter_context(tc.tile_pool(name="idx", bufs=1))
        pv_pool = ctx.enter_context(tc.tile_pool(name="pv", bufs=4))
        cen_pool = ctx.enter_context(tc.tile_pool(name="cen", bufs=1))
        neg_pool = ctx.enter_context(tc.tile_pool(name="neg", bufs=3))
        prod_pool = ctx.enter_context(tc.tile_pool(name="prod", bufs=1))
        red_pool = ctx.enter_context(tc.tile_pool(name="red", bufs=1))
        act_pool = ctx.enter_context(tc.tile_pool(name="act", bufs=2))
        out_pool = ctx.enter_context(tc.tile_pool(name="out", bufs=1))

        # ---- load all index tiles up front (small)
        PNF, PGF, NIX = [], [], []
        for v in range(V):
            tpn = idx_pool.tile([P, pc_cols], I32, tag=f"pnf{v}",
                                name=f"pnf{v}")
            nc.sync.dma_start(tpn[:], pnf.ap()[v])
            PNF.append(tpn)
            tpg = idx_pool.tile([P, pc_cols], I32, tag=f"pgf{v}",
                                name=f"pgf{v}")
            nc.sync.dma_start(tpg[:], pgf.ap()[v])
            PGF.append(tpg)
        for t in range(T):
            tn = idx_pool.tile([P, nc_cols], I32, tag=f"nidx{t}",
                               name=f"nidx{t}")
            nc.sync.dma_start(tn[:], nidx.ap()[t])
            NIX.append(tn)

        # ---- resident tiles
        CEN = [cen_pool.tile([P, pc_cols * D], F16, tag=f"cen{v}",
                             name=f"cen{v}") for v in range(V)]
        XN = [red_pool.tile([P, nc_cols], F32, tag=f"xn{t}", name=f"xn{t}")
              for t in range(T)]
        XPN = [red_pool.tile([P, pc_cols * V], F32, tag=f"xpn{v}",
                             name=f"xpn{v}") for v in range(V)]
        XPG = [red_pool.tile([P, pc_cols * V], F32, tag=f"xpg{v}",
                             name=f"xpg{v}") for v in range(V)]
        ACC = out_pool.tile([P, 2 * T], F32)

        for c in range(nchunk):
            jsl = slice(c * jb, (c + 1) * jb)
            for v in range(V):
                # node-fused gather: idx [P, jb] -> [P, jb*V*D]
                # (V*D contiguous elements per index: V view-rows of index n)
                PVNt = pv_pool.tile([P, jb * V * D], F16, tag="pv",
                                    name=f"pvn_{c}_{v}")
                for j in range(jb):
                    gather(PVNt[:, j * V * D:(j + 1) * V * D], W.ap(),
                           PNF[v][:, c * jb + j:c * jb + j + 1])
                pvn4 = PVNt[:].rearrange("p (j w d) -> p j w d", w=V, d=D)
                cen_src = pvn4[:, :, v, :]          # [P, jb, D] stride V*D
                csl = CEN[v][:, c * jb * D:(c + 1) * jb * D] \
                    .rearrange("p (j d) -> p j d", d=D)
                nc.scalar.activation(out=csl, in_=cen_src, func=CPY)
                # node-pos dots for all V components against center v
                prodp = prod_pool.tile([P, jb * V * D], F16, tag="prodp",
                                       name=f"prodp_{c}_{v}")
                pr4 = prodp[:].rearrange("p (j w d) -> p j w d", w=V, d=D)
                nc.vector.tensor_tensor(
                    out=pr4, in0=pvn4,
                    in1=cen_src.unsqueeze(2).to_broadcast([P, jb, V, D]),
                    op=MUL)
                nc.vector.tensor_reduce(
                    out=XPN[v][:, c * jb * V:(c + 1) * jb * V],
                    in_=pr4, axis=AXX, op=ADD)
                # neigh-fused gather + pos dots
                PVGt = pv_pool.tile([P, jb * V * D], F16, tag="pv",
                                    name=f"pvg_{c}_{v}")
                for j in range(jb):
                    gather(PVGt[:, j * V * D:(j + 1) * V * D], W.ap(),
                           PGF[v][:, c * jb + j:c * jb + j + 1])
                pvg4 = PVGt[:].rearrange("p (j w d) -> p j w d", w=V, d=D)
                prodg = prod_pool.tile([P, jb * V * D], F16, tag="prodp",
                                       name=f"prodg_{c}_{v}")
                pg4 = prodg[:].rearrange("p (j w d) -> p j w d", w=V, d=D)
                nc.vector.tensor_tensor(
                    out=pg4, in0=pvg4,
                    in1=csl.unsqueeze(2).to_broadcast([P, jb, V, D]),
                    op=MUL)
                nc.vector.tensor_reduce(
                    out=XPG[v][:, c * jb * V:(c + 1) * jb * V],
                    in_=pg4, axis=AXX, op=ADD)

            for t in range(T):
                iv = TERM_VIEW[t]
                NEG = neg_pool.tile([P, jb * k * D], F16, tag="neg",
                                    name=f"neg_{c}_{t}")
                for q in range(jb * k):
                    col = c * jb * k + q
                    gather(NEG[:, q * D:(q + 1) * D], W.ap(),
                           NIX[t][:, col:col + 1])
                neg4 = NEG[:].rearrange("p (j k d) -> p j k d", k=k, d=D)
                cenv = CEN[iv][:, c * jb * D:(c + 1) * jb * D] \
                    .rearrange("p (j d) -> p j d", d=D)
                prod = prod_pool.tile([P, jb * k * D], F16, tag="prod",
                                      name=f"prod_{c}_{t}")
                pr = prod[:].rearrange("p (j k d) -> p j k d", k=k, d=D)
                nc.vector.tensor_tensor(
                    out=pr, in0=neg4,
                    in1=cenv.unsqueeze(2).to_broadcast([P, jb, k, D]),
                    op=MUL)
                nc.vector.tensor_reduce(
                    out=XN[t][:, c * jb * k:(c + 1) * jb * k],
                    in_=pr, axis=AXX, op=ADD)

        # ---- log-sigmoid + per-term accumulation
        # neg sum: ln(sigmoid(-x)); pos sum: ln(sigmoid(x))
        for t in range(T):
            sgn = act_pool.tile([P, nc_cols], F32, tag="sgn", name=f"sgn{t}")
            nc.scalar.activation(out=sgn[:], in_=XN[t][:], func=SIG,
                                 scale=-1.0)
            spn = act_pool.tile([P, nc_cols], F32, tag="spn", name=f"spn{t}")
            nc.scalar.activation(out=spn[:], in_=sgn[:], func=LN,
                                 accum_out=ACC[:, t:t + 1])
            if t < V:
                src, col = XPG[t], t
            elif t < V + len(PAIRS):
                j, i = PAIRS[t - V]
                src, col = XPN[i], j
            else:
                j, i = PAIRS[t - V - len(PAIRS)]
                src, col = XPG[i], j
            pos_ap = src[:].rearrange("p (s w) -> p s w", w=V)[:, :, col]
            sgp = act_pool.tile([P, pc_cols], F32, tag="sgp", name=f"sgp{t}")
            nc.scalar.activation(out=sgp[:], in_=pos_ap, func=SIG)
            spp = act_pool.tile([P, pc_cols], F32, tag="spp", name=f"spp{t}")
            nc.scalar.activation(out=spp[:], in_=sgp[:], func=LN,
                                 accum_out=ACC[:, T + t:T + t + 1])

        nc.sync.dma_start(acc_out.ap(), ACC[:])

    nc.compile()
    return nc


_NC_CACHE = {}


def _get_nc(bc, k, nchunk):
    key = (bc, k, nchunk)
    if key not in _NC_CACHE:
        _NC_CACHE[key] = build_bass(bc, k, nchunk)
    return _NC_CACHE[key]


def _lay2(x, nchunk, jb):
    # x: [..., bc] -> [..., P, nchunk*jb] with (c,p,j) -> col c*jb+j on part p
    lead = x.shape[:-1]
    return (x.reshape(*lead, nchunk, P, jb)
             .swapaxes(-3, -2)
             .reshape(*lead, P, nchunk * jb))


def _lay3(x, nchunk, jb, k):
    # x: [..., bc, k] -> [..., P, nchunk*jb*k]
    lead = x.shape[:-2]
    return (x.reshape(*lead, nchunk, P, jb, k)
             .swapaxes(-4, -3)
             .reshape(*lead, P, nchunk * jb * k))


def host_prep(count, shuffle_indices, nodes_idx, neigh_idx,
              neg_idx1, neg_idx2, neg_idx3, node_W, neigh_W,
              n_cores=NCORES, nchunk=4, b=B):
    """Compute per-core input maps + the fused fp16 table. Pure numpy."""
    c0 = int(count)
    sh = np.asarray(shuffle_indices)[:, c0:c0 + b].astype(np.int64)
    nodes_sel = np.take_along_axis(
        np.asarray(nodes_idx).astype(np.int64), sh, axis=1)
    neigh_sel = np.take_along_axis(
        np.asarray(neigh_idx).astype(np.int64), sh, axis=1)
    neg1 = np.asarray(neg_idx1).astype(np.int64)[:, :b]
    neg2 = np.asarray(neg_idx2).astype(np.int64)[:, :, :b]
    neg3 = np.asarray(neg_idx3).astype(np.int64)[:, :, :b]

    # fused view-major interleave: node rows at n*V+v, neigh at N*V + n*V+v
    node16 = np.asarray(node_W).astype(np.float16)    # [V, N, D]
    neigh16 = np.asarray(neigh_W).astype(np.float16)
    W_all = np.empty((2 * N * V, D), dtype=np.float16)
    W_all[:N * V] = node16.transpose(1, 0, 2).reshape(N * V, D)
    W_all[N * V:] = neigh16.transpose(1, 0, 2).reshape(N * V, D)

    # per-term fused negative indices
    neg_list = []
    for i in range(V):
        neg_list.append(N * V + neg1[i] * V + i)
    for (j, i) in PAIRS:
        neg_list.append(neg2[j, i] * V + j)
    for (j, i) in PAIRS:
        neg_list.append(N * V + neg3[j, i] * V + j)
    neg_all = np.stack(neg_list)                      # [T, b, K]
    pnf_all = nodes_sel * V                           # [V, b]
    pgf_all = N * V + neigh_sel * V                   # [V, b]

    bc = b // n_cores
    chunk = bc // nchunk
    jb = chunk // P
    k = neg_all.shape[-1]

    in_maps = []
    for core in range(n_cores):
        sl = slice(core * bc, (core + 1) * bc)
        in_maps.append({
            "w_all": W_all,
            "pnf": _lay2(pnf_all[:, sl], nchunk, jb).astype(np.int32),
            "pgf": _lay2(pgf_all[:, sl], nchunk, jb).astype(np.int32),
            "nidx": _lay3(neg_all[:, sl], nchunk, jb, k).astype(np.int32),
        })
    return in_maps


def host_combine(acc_list, hyp1, hyp2, b=B):
    """acc_list: per-core [P, 2T] log-sigmoid-sum partials -> final scalar."""
    s = np.zeros(T, dtype=np.float64)
    for a in acc_list:
        a = np.asarray(a, dtype=np.float64).sum(axis=0)
        s += a[:T] + a[T:2 * T]
    term_val = s / b
    cost1 = term_val[0:3].mean()
    cost2 = float(np.asarray(hyp1).reshape(-1)[0]) * term_val[3:9].sum() / 6.0
    cost3 = float(np.asarray(hyp2).reshape(-1)[0]) * term_val[9:15].sum() / 6.0
    return np.array(-(cost1 + cost2 + cost3) / 3.0, dtype=np.float32)


def kernel(count, shuffle_indices, nodes_idx, neigh_idx,
           neg_idx1, neg_idx2, neg_idx3, node_W, neigh_W, hyp1, hyp2):
    in_maps = host_prep(count, shuffle_indices, nodes_idx, neigh_idx,
                        neg_idx1, neg_idx2, neg_idx3, node_W, neigh_W)
    nc = _get_nc(B // NCORES, K, 4)
    res = run_bass_kernel_spmd(nc, in_maps, core_ids=list(range(NCORES)))
    acc_list = [r["acc"] for r in res.results]
    return host_combine(acc_list, hyp1, hyp2)
